# revision 1
# baseline (speedup 1.0000x reference)
"""BeansBackboneV2 sparse-attention block on 8 TRN2 NeuronCores.

Sharding: data-parallel over batch B=2 (4 cores per batch group); within a
group the 16 attention heads are sharded 4 per core and the MLP hidden dim
(4096) is sharded 1024 per core.  The router (top-32 content routes) is
computed replicated within each group.  One on-device AllReduce (groups
[0-3], [4-7]) combines the head-sharded proj partials + residual; the final
hidden-sharded MLP partials are summed on the host during unsharding.

All activations flow in transposed [feature, token] layout so every linear
uses host-pre-transposed weights as the stationary matmul operand.  The
sparse gather attention is evaluated densely: top-32 selection runs on the
DVE max8/match-replace instructions, producing an additive bias matrix
(log route weight for selected pairs, -87 for the rest) shared by all
heads; attention is then dense matmuls + masked softmax — no gather DMA.

kernel(**inputs) takes the full unsharded inputs from setup_inputs() and
returns the full [2, 1025, 1024] output.
"""

import numpy as np

B, S, D, H, P = 2, 1025, 1024, 16, 1024
HD = D // H               # 64
HPC = 4                   # heads per core
DHC = HPC * HD            # 256 head-sharded feature cols per core
FFH = 1024                # hidden slice per core (4096 / 4)
TEMP = 0.1
SCALE = HD ** -0.5
EPS = 1e-5
EXCL = -87.0              # additive bias for non-routed pairs (exp -> ~1e-38)
NK = D // 128             # 8 contraction chunks
SBLK = [(0, 512), (512, 512), (1024, 1)]          # token blocks of S=1025
VOFF = {
    'norm1_w': 0, 'norm1_b': 8, 'rq_b': 16, 'rk_b': 24,
    'proj_b': 32, 'norm2_w': 40, 'norm2_b': 48, 'fc1_b': 56, 'fc2_b': 64,
    'qkv_bq': 72, 'qkv_bk': 74, 'qkv_bv': 76,     # [256] vecs -> 2 cols
}
NV = 78

_CACHE = {}


def build_nc(sim_gelu=False, reps=1, no_cc=False, phases=99):
    import concourse.bass as bass
    import concourse.bacc as bacc
    import concourse.mybir as mybir
    import concourse.tile as tile
    from concourse.masks import make_identity
    from contextlib import ExitStack

    f32 = mybir.dt.float32
    A = mybir.AluOpType
    AF = mybir.ActivationFunctionType
    X = mybir.AxisListType.X

    nc = bacc.Bacc("TRN2", target_bir_lowering=False, debug=False,
                   num_devices=8)
    f32r = mybir.dt.float32r

    def mm(out, lhsT, rhs, **kw):
        if rhs.free_size() % 2:
            return nc.tensor.matmul(out, lhsT, rhs, **kw)
        return nc.tensor.matmul(out, lhsT.bitcast(f32r), rhs.bitcast(f32r), **kw)

    def din(name, shape):
        return nc.declare_dram_parameter(name, list(shape), f32, isOutput=False)

    x_t = din("x_t", [D, S])
    rq_wT = din("rq_wT", [D, D])
    rk_wT = din("rk_wT", [D, D])
    pos_bias = din("pos_bias", [P, P])
    wqT = din("wqT", [D, DHC])
    wkT = din("wkT", [D, DHC])
    wvT = din("wvT", [D, DHC])
    projT = din("projT", [DHC, D])
    fc1T = din("fc1T", [D, FFH])
    fc2T = din("fc2T", [FFH, D])
    vecs = din("vecs", [128, NV])
    y_t = nc.declare_dram_parameter("y_t", [D, S], f32, isOutput=True)

    with tile.TileContext(nc) as tc:
      for _rep in range(reps):
        with ExitStack() as top:
                const = top.enter_context(tc.tile_pool(name="const", bufs=1))
                ones_raw = const.tile([128, 128], f32, tag="ones_raw", name="ones_raw")
                nc.vector.memset(ones_raw, 1.0)
                ones = const.tile([128, 128], f32, tag="ones", name="ones")
                nc.vector.tensor_copy(ones, ones_raw)
                zro = const.tile([128, 16], f32, tag="zro", name="zro")
                nc.vector.memset(zro, 0.0)
                ident = const.tile([128, 128], f32, tag="ident", name="ident")
                make_identity(nc, ident)
                vt = const.tile([128, NV], f32, tag="vt", name="vt")
                nc.sync.dma_start(vt, vecs[:, :])

                def vcol(key, m):
                    return vt[:, VOFF[key] + m:VOFF[key] + m + 1]

                # scaled copies: cols 0-7 proj_b*0.25, 8-15 fc2_b*0.25, 16-17 qkv_bq*SCALE
                sv = const.tile([128, 24], f32, tag="sv", name="sv")
                nc.vector.tensor_scalar_mul(sv[:, 0:8], vt[:, VOFF['proj_b']:VOFF['proj_b'] + 8], 0.25)
                nc.vector.tensor_scalar_mul(sv[:, 8:16], vt[:, VOFF['fc2_b']:VOFF['fc2_b'] + 8], 0.25)
                nc.vector.tensor_scalar_mul(sv[:, 16:18], vt[:, VOFF['qkv_bq']:VOFF['qkv_bq'] + 2], SCALE)

                stat = top.enter_context(tc.tile_pool(name="stat", bufs=1))
                scr = top.enter_context(tc.tile_pool(name="scr", bufs=2))

                # ---------------- helpers ----------------
                def layer_norm_T(src, dst_pool, wkey, bkey, tagp, out_f32r=False,
                                 stats_f32r=False):
                    """src: 8 x [128,S] transposed-activation tiles -> 8 normed tiles."""
                    with tc.tile_pool(name=f"lnp_{tagp}", bufs=2, space="PSUM") as lpp:
                        mean_b = stat.tile([128, S], f32, tag="mean_b", name="mean_b")
                        rstd_b = stat.tile([128, S], f32, tag="rstd_b", name="rstd_b")
                        for (soff, slen) in SBLK:
                            ps_s = lpp.tile([128, 512], f32, tag="ln_s", name="ps_s")
                            ps_q = lpp.tile([128, 512], f32, tag="ln_q", name="ps_q")
                            mx = mm if stats_f32r else nc.tensor.matmul
                            on = ones if stats_f32r else ones_raw
                            for c in range(NK):
                                sq = scr.tile([128, 512], f32, tag="sq", name="sq")
                                sqw = sq[:, :slen] if stats_f32r \
                                    else sq[:, :slen]
                                nc.scalar.activation(sqw,
                                                     src[c][:, soff:soff + slen], AF.Square)
                                mx(ps_s[:, :slen], on, src[c][:, soff:soff + slen],
                                   start=(c == 0), stop=(c == NK - 1))
                                mx(ps_q[:, :slen], on, sq[:, :slen],
                                   start=(c == 0), stop=(c == NK - 1))
                            m = mean_b[:, soff:soff + slen]
                            r = rstd_b[:, soff:soff + slen]
                            nc.vector.tensor_scalar_mul(m, ps_s[:, :slen], 1.0 / D)
                            nc.vector.tensor_scalar_mul(r, ps_q[:, :slen], 1.0 / D)  # E[x^2]
                            msq = scr.tile([128, 512], f32, tag="rs", name="msq")
                            nc.vector.tensor_mul(msq[:, :slen], m, m)
                            nc.vector.tensor_sub(r, r, msq[:, :slen])                # var
                            nc.vector.tensor_scalar_add(r, r, EPS)
                            nc.scalar.activation(r, r, AF.Sqrt)
                            nc.vector.reciprocal(r, r)
                        dst = []
                        for c in range(NK):
                            d = dst_pool.tile([128, S], f32, tag=f"{tagp}{c}", name=f"{tagp}{c}")
                            dw = d.bitcast(f32r) if out_f32r else d
                            nc.vector.tensor_sub(dw, src[c], mean_b)
                            nc.vector.tensor_mul(dw, d, rstd_b)
                            nc.vector.tensor_scalar(dw, d, vcol(wkey, c), vcol(bkey, c),
                                                    A.mult, A.add)
                            dst.append(d)
                        return dst

                def gemm_T(wT_dram, Mo, act, act_off, Sw, evict, wtag, wsplit=None,
                           use_f32r=True):
                    """evict(m, soff, slen, ps) receives psum with
                    (wT.T @ act[:, act_off+soff : ...])[m*128:(m+1)*128].
                    Weight chunk tiles are streamed in wsplit-wide groups to bound
                    SBUF residency for Mo=1024 GEMMs."""
                    if wsplit is None:
                        wsplit = 512 if Mo > 512 else Mo
                    with tc.tile_pool(name=f"wp_{wtag}", bufs=1) as wp, \
                         tc.tile_pool(name=f"gp_{wtag}", bufs=3, space="PSUM") as gpp:
                        for mg in range(Mo // wsplit):
                            wts = []
                            for c in range(NK):
                                w = wp.tile([128, wsplit], f32, tag=f"{wtag}{c}",
                                            name=f"{wtag}{c}_{mg}")
                                wsrc = wT_dram[c * 128:(c + 1) * 128,
                                               mg * wsplit:(mg + 1) * wsplit]
                                if use_f32r:
                                    nc.sync.dma_start(w.bitcast(f32r),
                                                      wsrc.bitcast(f32r))
                                else:
                                    nc.sync.dma_start(w, wsrc)
                                wts.append(w)
                            for ml in range(wsplit // 128):
                                m = mg * (wsplit // 128) + ml
                                for (soff, slen) in SBLK:
                                    if soff >= Sw:
                                        continue
                                    slen = min(slen, Sw - soff)
                                    ps = gpp.tile([128, 512], f32, tag="gp", name="ps")
                                    mmx = mm if use_f32r else nc.tensor.matmul
                                    for c in range(NK):
                                        mmx(
                                            ps[:, :slen], wts[c][:, ml * 128:(ml + 1) * 128],
                                            act[c][:, act_off + soff:act_off + soff + slen],
                                            start=(c == 0), stop=(c == NK - 1))
                                    evict(m, soff, slen, ps)

                def l2norm_T(tiles, n_cols):
                    with tc.tile_pool(name="l2p", bufs=2, space="PSUM") as l2p:
                        rinv = stat.tile([128, 1024], f32, tag="rinv", name="rinv")
                        for half in range(n_cols // 512):
                            hs = slice(half * 512, half * 512 + 512)
                            ps = l2p.tile([128, 512], f32, tag="l2", name="ps_l2")
                            for c in range(NK):
                                sq = scr.tile([128, 512], f32, tag="sq", name="sq2")
                                nc.scalar.activation(sq, tiles[c][:, hs], AF.Square)
                                nc.tensor.matmul(ps, ones_raw, sq,
                                             start=(c == 0), stop=(c == NK - 1))
                            r = rinv[:, hs]
                            nc.scalar.activation(r, ps, AF.Sqrt)
                            nc.vector.tensor_scalar_max(r, r, 1e-12)
                            nc.vector.reciprocal(r, r)
                        for c in range(NK):
                            nc.vector.tensor_mul(tiles[c], tiles[c], rinv)

                def _close_stacks():
                    for _s in (xn_scope, qkv_es, bias_es, ao_es):
                        _s.close()

                # pools whose tiles must outlive the xnT scope, in LIFO close order:
                # ao_pool (dies after proj) below bias_pool/qkvp (die after attention)
                ao_es = ExitStack()
                ao_pool = ao_es.enter_context(tc.tile_pool(name="ao_pool", bufs=1))
                bias_es = ExitStack()
                bias_pool = bias_es.enter_context(tc.tile_pool(name="bias_pool", bufs=1))
                qkv_es = ExitStack()
                qkvp = qkv_es.enter_context(tc.tile_pool(name="qkvp", bufs=1))

                # ---------------- phase 0/1: xT load + LN1 ----------------
                xn_scope = ExitStack()
                xn_pool = xn_scope.enter_context(tc.tile_pool(name="xn_pool", bufs=1))
                with tc.tile_pool(name="xt0", bufs=1) as xt0:
                    xT = []
                    for c in range(NK):
                        t = xt0.tile([128, S], f32, tag=f"xT{c}", name=f"xT{c}")
                        nc.sync.dma_start(t, x_t[c * 128:(c + 1) * 128, :])
                        xT.append(t)
                    xnT = layer_norm_T(xT, xn_pool, 'norm1_w', 'norm1_b', 'xnT')
                # xT tiles released here; x_t re-read at proj time.
                if phases <= 1:
                    _close_stacks()
                    continue

                # ---------------- phase 2: router projections + l2norm ----------------
                biasT = [bias_pool.tile([128, P], f32, tag=f"bT{c}", name=f"bT{c}")
                         for c in range(NK)]
                with ExitStack() as ph23:
                    rpool = ph23.enter_context(tc.tile_pool(name="rpool", bufs=1))
                    q_rT = [rpool.tile([128, P], f32, tag=f"qr{c}", name=f"qr{c}")
                            for c in range(NK)]
                    k_rT = [rpool.tile([128, P], f32, tag=f"kr{c}", name=f"kr{c}")
                            for c in range(NK)]

                    def ev_r(dst, bk):
                        def ev(m, soff, slen, ps):
                            nc.scalar.activation(dst[m][:, soff:soff + slen], ps[:, :slen],
                                                 AF.Identity, bias=vcol(bk, m))
                        return ev
                    gemm_T(rq_wT, D, xnT, 1, P, ev_r(q_rT, 'rq_b'), "wrq", wsplit=256,
                           use_f32r=False)
                    gemm_T(rk_wT, D, xnT, 1, P, ev_r(k_rT, 'rk_b'), "wrk", wsplit=256,
                           use_f32r=False)
                    l2norm_T(q_rT, P)
                    l2norm_T(k_rT, P)

                    # ---------- phase 3: scores, top-32, bias matrix + transpose ----------
                    with tc.tile_pool(name="bp", bufs=1) as bp, \
                         tc.tile_pool(name="scp", bufs=2, space="PSUM") as scp, \
                         tc.tile_pool(name="tp", bufs=2, space="PSUM") as tp:
                        for qb in range(8):
                            pb = bp.tile([128, P], f32, tag="pbt2", name="pb")
                            nc.sync.dma_start(pb, pos_bias[qb * 128:(qb + 1) * 128, :])
                            nc.vector.tensor_scalar_mul(pb, pb, 1.0 / TEMP)
                            tnat = bp.tile([128, P], f32, tag="tnat", name="tnat")
                            for nb in range(2):
                                ns = slice(nb * 512, nb * 512 + 512)
                                ps = scp.tile([128, 512], f32, tag="sc", name="ps_sc")
                                for c in range(NK):
                                    nc.tensor.matmul(
                                        ps, q_rT[c][:, qb * 128:(qb + 1) * 128],
                                        k_rT[c][:, ns],
                                        start=(c == 0), stop=(c == NK - 1))
                                nc.vector.scalar_tensor_tensor(tnat[:, ns], ps, 1.0 / TEMP,
                                                               pb[:, ns], A.mult, A.add)
                            dg = slice(qb * 128, qb * 128 + 128)
                            nc.vector.scalar_tensor_tensor(tnat[:, dg], ident, -1e10,
                                                           tnat[:, dg], A.mult, A.add)
                            # top-32 via 4 rounds of max8 + match_replace
                            t2 = bp.tile([128, P], f32, tag="pbt2", name="t2")
                            vals = bp.tile([128, 32], f32, tag="vals", name="vals")
                            src_mr = tnat
                            for r in range(4):
                                nc.vector.max(vals[:, r * 8:(r + 1) * 8], src_mr)
                                nc.vector.match_replace(t2, vals[:, r * 8:(r + 1) * 8],
                                                        src_mr, -1e30)
                                src_mr = t2
                            e32 = bp.tile([128, 32], f32, tag="e32", name="e32")
                            nc.scalar.activation(e32, vals, AF.Exp)
                            lse = bp.tile([128, 1], f32, tag="lse", name="lse")
                            nc.vector.tensor_reduce(lse, e32, X, A.add)
                            nc.scalar.activation(lse, lse, AF.Ln)
                            # bias = sel*(max(t-lse,-10) - EXCL) + EXCL, built in
                            # place on tnat; sel = (t2 < -1e20) fused into the STT.
                            bn = tnat
                            nc.vector.tensor_scalar(bn, tnat, lse[:, 0:1], -10.0,
                                                    A.subtract, A.max)
                            nc.vector.tensor_scalar_add(bn, bn, -EXCL)
                            nc.vector.scalar_tensor_tensor(bn, t2, -1e20, bn,
                                                           A.is_lt, A.mult)
                            nc.vector.tensor_scalar_add(bn, bn, EXCL)
                            for kb in range(8):
                                pt = tp.tile([128, 128], f32, tag="pt", name="pt")
                                nc.tensor.transpose(pt, bn[:, kb * 128:(kb + 1) * 128], ident)
                                nc.scalar.copy(biasT[kb][:, qb * 128:(qb + 1) * 128], pt)

                if phases <= 3:
                    _close_stacks()
                    continue
                # ---------------- phase 4: QKV for the 4 local heads ----------------
                QTs = [qkvp.tile([128, S], f32, tag=f"QT{i}", name=f"QT{i}") for i in range(2)]
                KTt = [qkvp.tile([128, S], f32, tag=f"KT{i}", name=f"KT{i}") for i in range(2)]
                Vn = [qkvp.tile([128, DHC], f32, tag=f"Vn{i}", name=f"Vn{i}") for i in range(9)]
                bv_row = qkvp.tile([1, DHC], f32, tag="bv_row", name="bv_row")

                def ev_q(m, soff, slen, ps):
                    nc.scalar.activation(QTs[m][:, soff:soff + slen],
                                         ps[:, :slen], AF.Identity,
                                         bias=sv[:, 16 + m:17 + m], scale=SCALE)

                def ev_k(m, soff, slen, ps):
                    nc.scalar.activation(KTt[m][:, soff:soff + slen],
                                         ps[:, :slen], AF.Identity, bias=vcol('qkv_bk', m))
                gemm_T(wqT, DHC, xnT, 0, S, ev_q, "wq", use_f32r=False)
                gemm_T(wkT, DHC, xnT, 0, S, ev_k, "wk", use_f32r=False)

                # qkv_bv as a [1, 256] row (PE transpose of the packed columns + 2 DMAs)
                with tc.tile_pool(name="vbp", bufs=1) as vbp, \
                     tc.tile_pool(name="vbps", bufs=1, space="PSUM") as vbps:
                    ptv = vbps.tile([128, 128], f32, tag="ptv", name="ptv")
                    nc.tensor.transpose(ptv[0:2, :], vt[:, VOFF['qkv_bv']:VOFF['qkv_bv'] + 2],
                                        ident)
                    s2 = vbp.tile([2, 128], f32, tag="s2", name="s2")
                    nc.scalar.copy(s2, ptv[0:2, :])
                    nc.sync.dma_start(bv_row[0:1, 0:128], s2[0:1, :])
                    nc.sync.dma_start(bv_row[0:1, 128:256], s2[1:2, :])

                with tc.tile_pool(name="wvp", bufs=1) as wvp, \
                     tc.tile_pool(name="vps", bufs=2, space="PSUM") as vpsp:
                    wvt = []
                    for c in range(NK):
                        w = wvp.tile([128, DHC], f32, tag=f"wv{c}", name=f"wv{c}")
                        nc.sync.dma_start(w, wvT[c * 128:(c + 1) * 128, :])
                        wvt.append(w)
                    vblocks = [(0, 1)] + [(1 + 128 * k, 128) for k in range(8)]
                    for vi, (voff, vlen) in enumerate(vblocks):
                        ps = vpsp.tile([128, DHC], f32, tag="vps", name="ps_v")
                        for c in range(NK):
                            nc.tensor.matmul(ps[:vlen, :], xnT[c][:, voff:voff + vlen],
                                             wvt[c], start=(c == 0), stop=False)
                        nc.tensor.matmul(ps[:vlen, :], ones_raw[0:1, 0:vlen], bv_row,
                                         start=False, stop=True)
                        nc.scalar.copy(Vn[vi][:vlen, :], ps[:vlen, :])
                xn_scope.close()  # xnT released

                if phases <= 4:
                    _close_stacks()
                    continue
                # ---------------- phase 5: attention ----------------
                aoutT = [ao_pool.tile([64, S], f32, tag=f"ao{i}", name=f"ao{i}")
                         for i in range(4)]
                with tc.tile_pool(name="ep", bufs=4) as ep, \
                     tc.tile_pool(name="spp", bufs=3, space="PSUM") as spp, \
                     tc.tile_pool(name="pop", bufs=2, space="PSUM") as pop, \
                     tc.tile_pool(name="dnp", bufs=2, space="PSUM") as dnp:
                    for hl in range(HPC):
                        ti, ro = hl // 2, (hl % 2) * 64
                        rs = slice(ro, ro + 64)
                        hc = slice(64 * hl, 64 * hl + 64)
                        QTh = QTs[ti][rs, :]
                        KTh = KTt[ti][rs, :]
                        # --- CLS query (token 0) over all S keys (fp32, tiny) ---
                        ecl = ep.tile([128, 16], f32, tag="ecl", name="ecl")
                        nc.vector.tensor_copy(ecl[:, 0:10], zro[:, 0:10])
                        pc = spp.tile([128, 512], f32, tag="sp", name="pc")
                        nc.tensor.matmul(pc[0:1, 0:1], KTh[:, 0:1], QTh[:, 0:1],
                                         start=True, stop=True)
                        nc.scalar.activation(ecl[0:1, 0:1], pc[0:1, 0:1],
                                             AF.Exp)
                        for j in range(8):
                            ks = slice(1 + 128 * j, 1 + 128 * (j + 1))
                            nc.tensor.matmul(pc[:, 1 + j:2 + j], KTh[:, ks], QTh[:, 0:1],
                                             start=True, stop=True)
                            nc.scalar.activation(ecl[:, 1 + j:2 + j],
                                                 pc[:, 1 + j:2 + j], AF.Exp)
                        dn = dnp.tile([64, 512], f32, tag="dn", name="dnc")
                        nc.tensor.matmul(dn[:, 0:10], ones_raw[:, 0:64],
                                         ecl[:, 0:10], start=True, stop=True)
                        dsum = ep.tile([64, 2], f32, tag="dsum", name="dsum")
                        nc.vector.tensor_reduce(dsum[:, 0:1], dn[:, 0:10], X, A.add)
                        nc.vector.reciprocal(dsum[:, 0:1], dsum[:, 0:1])
                        po = pop.tile([64, 512], f32, tag="po", name="poc")
                        nc.tensor.matmul(po[:, 0:1], Vn[0][0:1, hc],
                                         ecl[0:1, 0:1], start=True, stop=False)
                        for j in range(8):
                            nc.tensor.matmul(po[:, 0:1], Vn[1 + j][:, hc],
                                             ecl[:, 1 + j:2 + j],
                                             start=False, stop=(j == 7))
                        nc.vector.tensor_scalar_mul(aoutT[hl][:, 0:1],
                                                    po[:, 0:1], dsum[:, 0:1])
                        # --- patch queries, 2 blocks of 512 ---
                        for qs in range(2):
                            qcol = slice(1 + qs * 512, 1 + qs * 512 + 512)
                            bcol = slice(qs * 512, qs * 512 + 512)
                            po = pop.tile([64, 512], f32, tag="po", name="pop_")
                            dn = dnp.tile([64, 512], f32, tag="dn", name="dnp_")
                            for kb in range(8):
                                ks = slice(1 + 128 * kb, 1 + 128 * (kb + 1))
                                sp = spp.tile([128, 512], f32, tag="sp", name="sp_")
                                nc.tensor.matmul(sp, KTh[:, ks], QTh[:, qcol], start=True, stop=True)
                                ek = ep.tile([128, 512], f32, tag="ek", name="ek")
                                nc.vector.tensor_add(ek, sp,
                                                     biasT[kb][:, bcol])
                                nc.scalar.activation(ek, ek, AF.Exp)
                                nc.tensor.matmul(po, Vn[1 + kb][:, hc], ek,
                                   start=(kb == 0), stop=(kb == 7))
                                nc.tensor.matmul(dn, ones[:, 0:64], ek,
                                   start=(kb == 0), stop=(kb == 7))
                            rec = ep.tile([64, 512], f32, tag="rec", name="rec")
                            nc.vector.reciprocal(rec, dn)
                            nc.vector.tensor_mul(aoutT[hl][:, qcol],
                                                 po, rec)

                qkv_es.close()
                bias_es.close()

                # ---------------- phase 6: proj partial + residual/4 -> AllReduce ----------
                dram = top.enter_context(tc.tile_pool(name="dram", bufs=1, space="DRAM"))
                ar_in = dram.tile([D, S], f32, tag="ar_in", name="ar_in")
                ar_out = dram.tile([D, S], f32, tag="ar_out", name="ar_out")
                with tc.tile_pool(name="pp", bufs=1) as pp, \
                     tc.tile_pool(name="arp", bufs=2) as arp, \
                     tc.tile_pool(name="xtr", bufs=2) as xtr, \
                     tc.tile_pool(name="pjp", bufs=3, space="PSUM") as pjp:
                    pts = []
                    for c in range(4):
                        w = pp.tile([64, D], f32, tag=f"pw{c}", name=f"pw{c}")
                        nc.sync.dma_start(w,
                                          projT[c * 64:(c + 1) * 64, :])
                        pts.append(w)
                    for m in range(8):
                        xtm = xtr.tile([128, S], f32, tag="xtm", name="xtm")
                        nc.sync.dma_start(xtm, x_t[m * 128:(m + 1) * 128, :])
                        art = arp.tile([128, S], f32, tag="art", name="art")
                        for (soff, slen) in SBLK:
                            ps = pjp.tile([128, 512], f32, tag="pj", name="ps_p")
                            for c in range(4):
                                nc.tensor.matmul(ps[:, :slen], pts[c][:, m * 128:(m + 1) * 128],
                                   aoutT[c][:, soff:soff + slen],
                                   start=(c == 0), stop=(c == 3))
                            t = scr.tile([128, 512], f32, tag="rs", name="prs")
                            nc.vector.tensor_scalar(t[:, :slen], xtm[:, soff:soff + slen],
                                                    0.25, sv[:, m:m + 1], A.mult, A.add)
                            nc.vector.tensor_add(art[:, soff:soff + slen], t[:, :slen],
                                                 ps[:, :slen])
                        nc.sync.dma_start(ar_in[m * 128:(m + 1) * 128, :], art)
                nc.gpsimd.collective_compute(
                    "AllReduce", A.add, replica_groups=[[0, 1, 2, 3], [4, 5, 6, 7]],
                    ins=[ar_in.opt()], outs=[ar_out.opt()])

                ao_es.close()

                if phases <= 6:
                    _close_stacks()
                    continue
                # ---------------- phase 7/8: LN2 + hidden-sharded MLP ----------------
                x2p = top.enter_context(tc.tile_pool(name="x2p", bufs=1))
                x2T = []
                for c in range(NK):
                    t = x2p.tile([128, S], f32, tag=f"x2T{c}", name=f"x2T{c}")
                    nc.sync.dma_start(t, ar_out[c * 128:(c + 1) * 128, :])
                    x2T.append(t)
                with ExitStack() as ph8:
                    lp = ph8.enter_context(tc.tile_pool(name="lp", bufs=1))
                    ln2T = layer_norm_T(x2T, lp, 'norm2_w', 'norm2_b', 'l2T', out_f32r=True)
                    hT = [lp.tile([128, S], f32, tag=f"hT{c}", name=f"hT{c}")
                          for c in range(NK)]

                    def ev_h(m, soff, slen, ps):
                        dst = hT[m][:, soff:soff + slen]
                        if not sim_gelu:
                            nc.scalar.activation(dst.bitcast(f32r), ps[:, :slen], AF.Gelu,
                                                 bias=vcol('fc1_b', m))
                            return
                        # CoreSim has no Gelu LUT: tanh-approx composition (sim only)
                        nc.scalar.activation(dst, ps[:, :slen], AF.Identity,
                                             bias=vcol('fc1_b', m))
                        s1 = scr.tile([128, 512], f32, tag="gl1", name="s1")[:, :slen]
                        nc.scalar.activation(s1, dst, AF.Square)
                        nc.vector.tensor_scalar(s1, s1, 0.044715, 1.0, A.mult, A.add)
                        nc.vector.tensor_mul(s1, s1, dst)
                        nc.vector.tensor_scalar_mul(s1, s1, 0.7978845608028654)
                        nc.scalar.activation(s1, s1, AF.Tanh)
                        nc.vector.tensor_scalar(s1, s1, 1.0, 0.5, A.add, A.mult)
                        nc.vector.tensor_mul(dst, dst, s1)
                    gemm_T(fc1T, FFH, ln2T, 0, S, ev_h, "w1")

                    with tc.tile_pool(name="yp", bufs=2) as yp:
                        ytiles = {}

                        def ev_y(m, soff, slen, ps):
                            if m not in ytiles:
                                ytiles[m] = yp.tile([128, S], f32, tag="yt", name="yt")
                            t = scr.tile([128, 512], f32, tag="rs", name="yrs")
                            nc.vector.tensor_scalar(t[:, :slen], x2T[m][:, soff:soff + slen],
                                                    0.25, sv[:, 8 + m:9 + m], A.mult, A.add)
                            nc.vector.tensor_add(ytiles[m][:, soff:soff + slen], t[:, :slen],
                                                 ps[:, :slen])
                            if soff + slen >= S:
                                nc.sync.dma_start(y_t[m * 128:(m + 1) * 128, :], ytiles[m])
                        gemm_T(fc2T, D, hT, 0, S, ev_y, "w2")

    nc.compile()
    return nc


def _prep_in_maps(inputs):
    def c(a):
        return np.ascontiguousarray(np.asarray(a), dtype=np.float32)

    def rnd(a):
        # round to fp32r-representable values (zero low mantissa bits)
        return ((a.view(np.uint32) + 0x8000) & 0xFFFF0000).view(np.float32)

    qkv_w = np.asarray(inputs['qkv_w'])
    qkv_b = np.asarray(inputs['qkv_b'])
    in_maps = []
    for core in range(8):
        b, g = core // 4, core % 4
        hs = slice(4 * g * HD, 4 * g * HD + DHC)
        v = np.zeros((128, NV), np.float32)
        for k in ('norm1_w', 'norm1_b', 'rq_b', 'rk_b', 'proj_b',
                  'norm2_w', 'norm2_b', 'fc2_b'):
            arr = np.asarray(inputs[k])
            v[:, VOFF[k]:VOFF[k] + 8] = arr.reshape(8, 128).T
        v[:, VOFF['fc1_b']:VOFF['fc1_b'] + 8] = \
            np.asarray(inputs['fc1_b'])[FFH * g:FFH * (g + 1)].reshape(8, 128).T
        v[:, VOFF['qkv_bq']:VOFF['qkv_bq'] + 2] = qkv_b[0:D][hs].reshape(2, 128).T
        v[:, VOFF['qkv_bk']:VOFF['qkv_bk'] + 2] = qkv_b[D:2 * D][hs].reshape(2, 128).T
        v[:, VOFF['qkv_bv']:VOFF['qkv_bv'] + 2] = qkv_b[2 * D:][hs].reshape(2, 128).T
        in_maps.append({
            'x_t': c(np.asarray(inputs['x'])[b].T),
            'rq_wT': c(np.asarray(inputs['rq_w']).T),
            'rk_wT': c(np.asarray(inputs['rk_w']).T),
            'pos_bias': c(inputs['pos_bias']),
            'wqT': c(qkv_w[0:D][hs, :].T),
            'wkT': c(qkv_w[D:2 * D][hs, :].T),
            'wvT': c(qkv_w[2 * D:][hs, :].T),
            'projT': c(np.asarray(inputs['proj_w'])[:, hs].T),
            'fc1T': rnd(c(np.asarray(inputs['fc1_w'])[FFH * g:FFH * (g + 1), :].T)),
            'fc2T': rnd(c(np.asarray(inputs['fc2_w'])[:, FFH * g:FFH * (g + 1)].T)),
            'vecs': c(v),
        })
    return in_maps


def get_nc(sim_gelu=False, reps=1, no_cc=False, phases=99):
    key = f'nc{sim_gelu}_{reps}_{no_cc}_{phases}'
    if key not in _CACHE:
        _CACHE[key] = build_nc(sim_gelu, reps, no_cc, phases)
    return _CACHE[key]


def assemble(results):
    out = np.zeros((B, S, D), np.float32)
    for b in range(2):
        acc = np.zeros((D, S), np.float64)
        for c in range(4 * b, 4 * b + 4):
            acc += results[c]['y_t']
        out[b] = acc.T.astype(np.float32)
    return out


def kernel(**inputs):
    from concourse.bass_utils import run_bass_kernel_spmd
    nc = get_nc()
    in_maps = _prep_in_maps(inputs)
    res = run_bass_kernel_spmd(nc, in_maps, list(range(8))).results
    return assemble(res)



# revision 2
# speedup vs baseline: 216.0480x; 216.0480x over previous
"""BeansBackboneV2 sparse-attention block on 8 TRN2 NeuronCores.

Sharding: data-parallel over batch B=2 (4 cores per batch group); within a
group the 16 attention heads are sharded 4 per core and the MLP hidden dim
(4096) is sharded 1024 per core.  The router (top-32 content routes) is
computed replicated within each group.  One on-device AllReduce (groups
[0-3], [4-7]) combines the head-sharded proj partials + residual; the final
hidden-sharded MLP partials are summed on the host during unsharding.

All activations flow in transposed [feature, token] layout so every linear
uses host-pre-transposed weights as the stationary matmul operand.  The
sparse gather attention is evaluated densely: top-32 selection runs on the
DVE max8/match-replace instructions, producing an additive bias matrix
(log route weight for selected pairs, -87 for the rest) shared by all
heads; attention is then dense matmuls + masked softmax — no gather DMA.

kernel(**inputs) takes the full unsharded inputs from setup_inputs() and
returns the full [2, 1025, 1024] output.
"""

import numpy as np

B, S, D, H, P = 2, 1025, 1024, 16, 1024
HD = D // H               # 64
HPC = 4                   # heads per core
DHC = HPC * HD            # 256 head-sharded feature cols per core
FFH = 1024                # hidden slice per core (4096 / 4)
TEMP = 0.1
SCALE = HD ** -0.5
EPS = 1e-5
EXCL = -87.0              # additive bias for non-routed pairs (exp -> ~1e-38)
NK = D // 128             # 8 contraction chunks
SBLK = [(0, 512), (512, 512), (1024, 1)]          # token blocks of S=1025
VOFF = {
    'norm1_w': 0, 'norm1_b': 8, 'rq_b': 16, 'rk_b': 24,
    'proj_b': 32, 'norm2_w': 40, 'norm2_b': 48, 'fc1_b': 56, 'fc2_b': 64,
    'qkv_bq': 72, 'qkv_bk': 74, 'qkv_bv': 76,     # [256] vecs -> 2 cols
}
NV = 78

_CACHE = {}


def build_nc(sim_gelu=False, reps=1, no_cc=False, phases=99):
    import concourse.bass as bass
    import concourse.bacc as bacc
    import concourse.mybir as mybir
    import concourse.tile as tile
    from concourse.masks import make_identity
    from contextlib import ExitStack

    f32 = mybir.dt.float32
    A = mybir.AluOpType
    AF = mybir.ActivationFunctionType
    X = mybir.AxisListType.X

    nc = bacc.Bacc("TRN2", target_bir_lowering=False, debug=False,
                   num_devices=8)
    f32r = mybir.dt.float32r

    def mm(out, lhsT, rhs, **kw):
        if rhs.free_size() % 2:
            return nc.tensor.matmul(out, lhsT, rhs, **kw)
        return nc.tensor.matmul(out, lhsT.bitcast(f32r), rhs.bitcast(f32r), **kw)

    def din(name, shape):
        return nc.declare_dram_parameter(name, list(shape), f32, isOutput=False)

    x_t = din("x_t", [D, S])
    rq_wT = din("rq_wT", [D, D])
    rk_wT = din("rk_wT", [D, D])
    pos_bias = din("pos_bias", [P, P])
    wqT = din("wqT", [D, DHC])
    wkT = din("wkT", [D, DHC])
    wvT = din("wvT", [D, DHC])
    projT = din("projT", [DHC, D])
    fc1T = din("fc1T", [D, FFH])
    fc2T = din("fc2T", [FFH, D])
    vecs = din("vecs", [128, NV])
    y_t = nc.declare_dram_parameter("y_t", [D, S], f32, isOutput=True)

    with tile.TileContext(nc) as tc:
      for _rep in range(reps):
        with ExitStack() as top:
                const = top.enter_context(tc.tile_pool(name="const", bufs=1))
                ones_raw = const.tile([128, 128], f32, tag="ones_raw", name="ones_raw")
                nc.vector.memset(ones_raw, 1.0)
                ones = const.tile([128, 128], f32, tag="ones", name="ones")
                nc.vector.tensor_copy(ones, ones_raw)
                zro = const.tile([128, 16], f32, tag="zro", name="zro")
                nc.vector.memset(zro, 0.0)
                ident = const.tile([128, 128], f32, tag="ident", name="ident")
                make_identity(nc, ident)
                vt = const.tile([128, NV], f32, tag="vt", name="vt")
                nc.sync.dma_start(vt, vecs[:, :])

                def vcol(key, m):
                    return vt[:, VOFF[key] + m:VOFF[key] + m + 1]

                # scaled copies: cols 0-7 proj_b*0.25, 8-15 fc2_b*0.25, 16-17 qkv_bq*SCALE
                sv = const.tile([128, 24], f32, tag="sv", name="sv")
                nc.vector.tensor_scalar_mul(sv[:, 0:8], vt[:, VOFF['proj_b']:VOFF['proj_b'] + 8], 0.25)
                nc.vector.tensor_scalar_mul(sv[:, 8:16], vt[:, VOFF['fc2_b']:VOFF['fc2_b'] + 8], 0.25)
                nc.vector.tensor_scalar_mul(sv[:, 16:18], vt[:, VOFF['qkv_bq']:VOFF['qkv_bq'] + 2], SCALE)

                stat = top.enter_context(tc.tile_pool(name="stat", bufs=1))
                scr = top.enter_context(tc.tile_pool(name="scr", bufs=2))

                # ---------------- helpers ----------------
                def layer_norm_T(src, dst_pool, wkey, bkey, tagp, out_f32r=False,
                                 stats_f32r=False):
                    """src: 8 x [128,S] transposed-activation tiles -> 8 normed tiles."""
                    with tc.tile_pool(name=f"lnp_{tagp}", bufs=2, space="PSUM") as lpp:
                        mean_b = stat.tile([128, S], f32, tag="mean_b", name="mean_b")
                        rstd_b = stat.tile([128, S], f32, tag="rstd_b", name="rstd_b")
                        for (soff, slen) in SBLK:
                            ps_s = lpp.tile([128, 512], f32, tag="ln_s", name="ps_s")
                            ps_q = lpp.tile([128, 512], f32, tag="ln_q", name="ps_q")
                            mx = mm if stats_f32r else nc.tensor.matmul
                            on = ones if stats_f32r else ones_raw
                            for c in range(NK):
                                sq = scr.tile([128, 512], f32, tag="sq", name="sq")
                                sqw = sq[:, :slen] if stats_f32r \
                                    else sq[:, :slen]
                                nc.scalar.activation(sqw,
                                                     src[c][:, soff:soff + slen], AF.Square)
                                mx(ps_s[:, :slen], on, src[c][:, soff:soff + slen],
                                   start=(c == 0), stop=(c == NK - 1))
                                mx(ps_q[:, :slen], on, sq[:, :slen],
                                   start=(c == 0), stop=(c == NK - 1))
                            m = mean_b[:, soff:soff + slen]
                            r = rstd_b[:, soff:soff + slen]
                            nc.vector.tensor_scalar_mul(m, ps_s[:, :slen], 1.0 / D)
                            nc.vector.tensor_scalar_mul(r, ps_q[:, :slen], 1.0 / D)  # E[x^2]
                            msq = scr.tile([128, 512], f32, tag="rs", name="msq")
                            nc.vector.tensor_mul(msq[:, :slen], m, m)
                            nc.vector.tensor_sub(r, r, msq[:, :slen])                # var
                            nc.vector.tensor_scalar_add(r, r, EPS)
                            nc.scalar.activation(r, r, AF.Sqrt)
                            nc.vector.reciprocal(r, r)
                        dst = []
                        for c in range(NK):
                            d = dst_pool.tile([128, S], f32, tag=f"{tagp}{c}", name=f"{tagp}{c}")
                            dw = d.bitcast(f32r) if out_f32r else d
                            nc.vector.tensor_sub(dw, src[c], mean_b)
                            nc.vector.tensor_mul(dw, d, rstd_b)
                            nc.vector.tensor_scalar(dw, d, vcol(wkey, c), vcol(bkey, c),
                                                    A.mult, A.add)
                            dst.append(d)
                        return dst

                def gemm_T(wT_dram, Mo, act, act_off, Sw, evict, wtag, wsplit=None,
                           use_f32r=True):
                    """evict(m, soff, slen, ps) receives psum with
                    (wT.T @ act[:, act_off+soff : ...])[m*128:(m+1)*128].
                    Weight chunk tiles are streamed in wsplit-wide groups to bound
                    SBUF residency for Mo=1024 GEMMs."""
                    if wsplit is None:
                        wsplit = 512 if Mo > 512 else Mo
                    with tc.tile_pool(name=f"wp_{wtag}", bufs=1) as wp, \
                         tc.tile_pool(name=f"gp_{wtag}", bufs=3, space="PSUM") as gpp:
                        for mg in range(Mo // wsplit):
                            wts = []
                            for c in range(NK):
                                w = wp.tile([128, wsplit], f32, tag=f"{wtag}{c}",
                                            name=f"{wtag}{c}_{mg}")
                                wsrc = wT_dram[c * 128:(c + 1) * 128,
                                               mg * wsplit:(mg + 1) * wsplit]
                                if use_f32r:
                                    nc.sync.dma_start(w.bitcast(f32r),
                                                      wsrc.bitcast(f32r))
                                else:
                                    nc.sync.dma_start(w, wsrc)
                                wts.append(w)
                            for ml in range(wsplit // 128):
                                m = mg * (wsplit // 128) + ml
                                for (soff, slen) in SBLK:
                                    if soff >= Sw:
                                        continue
                                    slen = min(slen, Sw - soff)
                                    ps = gpp.tile([128, 512], f32, tag="gp", name="ps")
                                    mmx = mm if use_f32r else nc.tensor.matmul
                                    for c in range(NK):
                                        mmx(
                                            ps[:, :slen], wts[c][:, ml * 128:(ml + 1) * 128],
                                            act[c][:, act_off + soff:act_off + soff + slen],
                                            start=(c == 0), stop=(c == NK - 1))
                                    evict(m, soff, slen, ps)

                def l2norm_T(tiles, n_cols):
                    with tc.tile_pool(name="l2p", bufs=2, space="PSUM") as l2p:
                        rinv = stat.tile([128, 1024], f32, tag="rinv", name="rinv")
                        for half in range(n_cols // 512):
                            hs = slice(half * 512, half * 512 + 512)
                            ps = l2p.tile([128, 512], f32, tag="l2", name="ps_l2")
                            for c in range(NK):
                                sq = scr.tile([128, 512], f32, tag="sq", name="sq2")
                                nc.scalar.activation(sq, tiles[c][:, hs], AF.Square)
                                nc.tensor.matmul(ps, ones_raw, sq,
                                             start=(c == 0), stop=(c == NK - 1))
                            r = rinv[:, hs]
                            nc.scalar.activation(r, ps, AF.Sqrt)
                            nc.vector.tensor_scalar_max(r, r, 1e-12)
                            nc.vector.reciprocal(r, r)
                        for c in range(NK):
                            nc.vector.tensor_mul(tiles[c], tiles[c], rinv)

                def _close_stacks():
                    for _s in (xn_scope, qkv_es, bias_es, ao_es):
                        _s.close()

                # pools whose tiles must outlive the xnT scope, in LIFO close order:
                # ao_pool (dies after proj) below bias_pool/qkvp (die after attention)
                ao_es = ExitStack()
                ao_pool = ao_es.enter_context(tc.tile_pool(name="ao_pool", bufs=1))
                bias_es = ExitStack()
                bias_pool = bias_es.enter_context(tc.tile_pool(name="bias_pool", bufs=1))
                qkv_es = ExitStack()
                qkvp = qkv_es.enter_context(tc.tile_pool(name="qkvp", bufs=1))

                # ---------------- phase 0/1: xT load + LN1 ----------------
                xn_scope = ExitStack()
                xn_pool = xn_scope.enter_context(tc.tile_pool(name="xn_pool", bufs=1))
                with tc.tile_pool(name="xt0", bufs=1) as xt0:
                    xT = []
                    for c in range(NK):
                        t = xt0.tile([128, S], f32, tag=f"xT{c}", name=f"xT{c}")
                        nc.sync.dma_start(t, x_t[c * 128:(c + 1) * 128, :])
                        xT.append(t)
                    xnT = layer_norm_T(xT, xn_pool, 'norm1_w', 'norm1_b', 'xnT')
                # xT tiles released here; x_t re-read at proj time.
                if phases <= 1:
                    _close_stacks()
                    continue

                # ---------------- phase 2: router projections + l2norm ----------------
                biasT = [bias_pool.tile([128, P], f32, tag=f"bT{c}", name=f"bT{c}")
                         for c in range(NK)]
                with ExitStack() as ph23:
                    rpool = ph23.enter_context(tc.tile_pool(name="rpool", bufs=1))
                    q_rT = [rpool.tile([128, P], f32, tag=f"qr{c}", name=f"qr{c}")
                            for c in range(NK)]
                    k_rT = [rpool.tile([128, P], f32, tag=f"kr{c}", name=f"kr{c}")
                            for c in range(NK)]

                    def ev_r(dst, bk):
                        def ev(m, soff, slen, ps):
                            nc.scalar.activation(dst[m][:, soff:soff + slen], ps[:, :slen],
                                                 AF.Identity, bias=vcol(bk, m))
                        return ev
                    gemm_T(rq_wT, D, xnT, 1, P, ev_r(q_rT, 'rq_b'), "wrq", wsplit=256,
                           use_f32r=False)
                    gemm_T(rk_wT, D, xnT, 1, P, ev_r(k_rT, 'rk_b'), "wrk", wsplit=256,
                           use_f32r=False)
                    l2norm_T(q_rT, P)
                    l2norm_T(k_rT, P)

                    # ---------- phase 3: scores, top-32, bias matrix + transpose ----------
                    with tc.tile_pool(name="bp", bufs=1) as bp, \
                         tc.tile_pool(name="scp", bufs=2, space="PSUM") as scp, \
                         tc.tile_pool(name="tp", bufs=2, space="PSUM") as tp:
                        for qb in range(8):
                            pb = bp.tile([128, P], f32, tag="pbt2", name="pb")
                            nc.sync.dma_start(pb, pos_bias[qb * 128:(qb + 1) * 128, :])
                            nc.vector.tensor_scalar_mul(pb, pb, 1.0 / TEMP)
                            tnat = bp.tile([128, P], f32, tag="tnat", name="tnat")
                            for nb in range(2):
                                ns = slice(nb * 512, nb * 512 + 512)
                                ps = scp.tile([128, 512], f32, tag="sc", name="ps_sc")
                                for c in range(NK):
                                    nc.tensor.matmul(
                                        ps, q_rT[c][:, qb * 128:(qb + 1) * 128],
                                        k_rT[c][:, ns],
                                        start=(c == 0), stop=(c == NK - 1))
                                nc.vector.scalar_tensor_tensor(tnat[:, ns], ps, 1.0 / TEMP,
                                                               pb[:, ns], A.mult, A.add)
                            dg = slice(qb * 128, qb * 128 + 128)
                            nc.vector.scalar_tensor_tensor(tnat[:, dg], ident, -1e10,
                                                           tnat[:, dg], A.mult, A.add)
                            # top-32 via 4 rounds of max8 + match_replace
                            t2 = bp.tile([128, P], f32, tag="pbt2", name="t2")
                            vals = bp.tile([128, 32], f32, tag="vals", name="vals")
                            src_mr = tnat
                            for r in range(4):
                                nc.vector.max(vals[:, r * 8:(r + 1) * 8], src_mr)
                                nc.vector.match_replace(t2, vals[:, r * 8:(r + 1) * 8],
                                                        src_mr, -1e30)
                                src_mr = t2
                            e32 = bp.tile([128, 32], f32, tag="e32", name="e32")
                            nc.scalar.activation(e32, vals, AF.Exp)
                            lse = bp.tile([128, 1], f32, tag="lse", name="lse")
                            nc.vector.tensor_reduce(lse, e32, X, A.add)
                            nc.scalar.activation(lse, lse, AF.Ln)
                            # bias = sel*(max(t-lse,-10) - EXCL) + EXCL, built in
                            # place on tnat; sel = (t2 < -1e20) fused into the STT.
                            bn = tnat
                            nc.vector.tensor_scalar(bn, tnat, lse[:, 0:1], -10.0,
                                                    A.subtract, A.max)
                            nc.vector.tensor_scalar_add(bn, bn, -EXCL)
                            nc.vector.scalar_tensor_tensor(bn, t2, -1e20, bn,
                                                           A.is_lt, A.mult)
                            nc.vector.tensor_scalar_add(bn, bn, EXCL)
                            for kb in range(8):
                                pt = tp.tile([128, 128], f32, tag="pt", name="pt")
                                nc.tensor.transpose(pt, bn[:, kb * 128:(kb + 1) * 128], ident)
                                nc.scalar.copy(biasT[kb][:, qb * 128:(qb + 1) * 128], pt)

                if phases <= 3:
                    _close_stacks()
                    continue
                # ---------------- phase 4: QKV for the 4 local heads ----------------
                QTs = [qkvp.tile([128, S], f32, tag=f"QT{i}", name=f"QT{i}") for i in range(2)]
                KTt = [qkvp.tile([128, S], f32, tag=f"KT{i}", name=f"KT{i}") for i in range(2)]
                Vn = [qkvp.tile([128, DHC], f32, tag=f"Vn{i}", name=f"Vn{i}") for i in range(9)]
                bv_row = qkvp.tile([1, DHC], f32, tag="bv_row", name="bv_row")

                def ev_q(m, soff, slen, ps):
                    nc.scalar.activation(QTs[m][:, soff:soff + slen],
                                         ps[:, :slen], AF.Identity,
                                         bias=sv[:, 16 + m:17 + m], scale=SCALE)

                def ev_k(m, soff, slen, ps):
                    nc.scalar.activation(KTt[m][:, soff:soff + slen],
                                         ps[:, :slen], AF.Identity, bias=vcol('qkv_bk', m))
                gemm_T(wqT, DHC, xnT, 0, S, ev_q, "wq", use_f32r=False)
                gemm_T(wkT, DHC, xnT, 0, S, ev_k, "wk", use_f32r=False)

                # qkv_bv as a [1, 256] row (PE transpose of the packed columns + 2 DMAs)
                with tc.tile_pool(name="vbp", bufs=1) as vbp, \
                     tc.tile_pool(name="vbps", bufs=1, space="PSUM") as vbps:
                    ptv = vbps.tile([128, 128], f32, tag="ptv", name="ptv")
                    nc.tensor.transpose(ptv[0:2, :], vt[:, VOFF['qkv_bv']:VOFF['qkv_bv'] + 2],
                                        ident)
                    s2 = vbp.tile([2, 128], f32, tag="s2", name="s2")
                    nc.scalar.copy(s2, ptv[0:2, :])
                    nc.sync.dma_start(bv_row[0:1, 0:128], s2[0:1, :])
                    nc.sync.dma_start(bv_row[0:1, 128:256], s2[1:2, :])

                with tc.tile_pool(name="wvp", bufs=1) as wvp, \
                     tc.tile_pool(name="vps", bufs=2, space="PSUM") as vpsp:
                    wvt = []
                    for c in range(NK):
                        w = wvp.tile([128, DHC], f32, tag=f"wv{c}", name=f"wv{c}")
                        nc.sync.dma_start(w, wvT[c * 128:(c + 1) * 128, :])
                        wvt.append(w)
                    vblocks = [(0, 1)] + [(1 + 128 * k, 128) for k in range(8)]
                    for vi, (voff, vlen) in enumerate(vblocks):
                        ps = vpsp.tile([128, DHC], f32, tag="vps", name="ps_v")
                        for c in range(NK):
                            nc.tensor.matmul(ps[:vlen, :], xnT[c][:, voff:voff + vlen],
                                             wvt[c], start=(c == 0), stop=False)
                        nc.tensor.matmul(ps[:vlen, :], ones_raw[0:1, 0:vlen], bv_row,
                                         start=False, stop=True)
                        nc.scalar.copy(Vn[vi][:vlen, :], ps[:vlen, :])
                xn_scope.close()  # xnT released

                if phases <= 4:
                    _close_stacks()
                    continue
                # ---------------- phase 5: attention ----------------
                aoutT = [ao_pool.tile([64, S], f32, tag=f"ao{i}", name=f"ao{i}")
                         for i in range(4)]
                with tc.tile_pool(name="ep", bufs=4) as ep, \
                     tc.tile_pool(name="spp", bufs=3, space="PSUM") as spp, \
                     tc.tile_pool(name="pop", bufs=2, space="PSUM") as pop, \
                     tc.tile_pool(name="dnp", bufs=2, space="PSUM") as dnp:
                    for hl in range(HPC):
                        ti, ro = hl // 2, (hl % 2) * 64
                        rs = slice(ro, ro + 64)
                        hc = slice(64 * hl, 64 * hl + 64)
                        QTh = QTs[ti][rs, :]
                        KTh = KTt[ti][rs, :]
                        # --- CLS query (token 0) over all S keys (fp32, tiny) ---
                        ecl = ep.tile([128, 16], f32, tag="ecl", name="ecl")
                        nc.vector.tensor_copy(ecl[:, 0:10], zro[:, 0:10])
                        pc = spp.tile([128, 512], f32, tag="sp", name="pc")
                        nc.tensor.matmul(pc[0:1, 0:1], KTh[:, 0:1], QTh[:, 0:1],
                                         start=True, stop=True)
                        nc.scalar.activation(ecl[0:1, 0:1], pc[0:1, 0:1],
                                             AF.Exp)
                        for j in range(8):
                            ks = slice(1 + 128 * j, 1 + 128 * (j + 1))
                            nc.tensor.matmul(pc[:, 1 + j:2 + j], KTh[:, ks], QTh[:, 0:1],
                                             start=True, stop=True)
                            nc.scalar.activation(ecl[:, 1 + j:2 + j],
                                                 pc[:, 1 + j:2 + j], AF.Exp)
                        dn = dnp.tile([64, 512], f32, tag="dn", name="dnc")
                        nc.tensor.matmul(dn[:, 0:10], ones_raw[:, 0:64],
                                         ecl[:, 0:10], start=True, stop=True)
                        dsum = ep.tile([64, 2], f32, tag="dsum", name="dsum")
                        nc.vector.tensor_reduce(dsum[:, 0:1], dn[:, 0:10], X, A.add)
                        nc.vector.reciprocal(dsum[:, 0:1], dsum[:, 0:1])
                        po = pop.tile([64, 512], f32, tag="po", name="poc")
                        nc.tensor.matmul(po[:, 0:1], Vn[0][0:1, hc],
                                         ecl[0:1, 0:1], start=True, stop=False)
                        for j in range(8):
                            nc.tensor.matmul(po[:, 0:1], Vn[1 + j][:, hc],
                                             ecl[:, 1 + j:2 + j],
                                             start=False, stop=(j == 7))
                        nc.vector.tensor_scalar_mul(aoutT[hl][:, 0:1],
                                                    po[:, 0:1], dsum[:, 0:1])
                        # --- patch queries, 2 blocks of 512 ---
                        for qs in range(2):
                            qcol = slice(1 + qs * 512, 1 + qs * 512 + 512)
                            bcol = slice(qs * 512, qs * 512 + 512)
                            po = pop.tile([64, 512], f32, tag="po", name="pop_")
                            dn = dnp.tile([64, 512], f32, tag="dn", name="dnp_")
                            for kb in range(8):
                                ks = slice(1 + 128 * kb, 1 + 128 * (kb + 1))
                                sp = spp.tile([128, 512], f32, tag="sp", name="sp_")
                                nc.tensor.matmul(sp, KTh[:, ks], QTh[:, qcol], start=True, stop=True)
                                ek = ep.tile([128, 512], f32, tag="ek", name="ek")
                                nc.vector.tensor_add(ek, sp,
                                                     biasT[kb][:, bcol])
                                nc.scalar.activation(ek, ek, AF.Exp)
                                nc.tensor.matmul(po, Vn[1 + kb][:, hc], ek,
                                   start=(kb == 0), stop=(kb == 7))
                                nc.tensor.matmul(dn, ones[:, 0:64], ek,
                                   start=(kb == 0), stop=(kb == 7))
                            rec = ep.tile([64, 512], f32, tag="rec", name="rec")
                            nc.vector.reciprocal(rec, dn)
                            nc.vector.tensor_mul(aoutT[hl][:, qcol],
                                                 po, rec)

                qkv_es.close()
                bias_es.close()

                # ---------------- phase 6: proj partial + residual/4 -> AllReduce ----------
                dram = top.enter_context(tc.tile_pool(name="dram", bufs=1, space="DRAM"))
                ar_in = dram.tile([D, S], f32, tag="ar_in", name="ar_in")
                ar_out = dram.tile([D, S], f32, tag="ar_out", name="ar_out")
                with tc.tile_pool(name="pp", bufs=1) as pp, \
                     tc.tile_pool(name="arp", bufs=2) as arp, \
                     tc.tile_pool(name="xtr", bufs=2) as xtr, \
                     tc.tile_pool(name="pjp", bufs=3, space="PSUM") as pjp:
                    pts = []
                    for c in range(4):
                        w = pp.tile([64, D], f32, tag=f"pw{c}", name=f"pw{c}")
                        nc.sync.dma_start(w,
                                          projT[c * 64:(c + 1) * 64, :])
                        pts.append(w)
                    for m in range(8):
                        xtm = xtr.tile([128, S], f32, tag="xtm", name="xtm")
                        nc.sync.dma_start(xtm, x_t[m * 128:(m + 1) * 128, :])
                        art = arp.tile([128, S], f32, tag="art", name="art")
                        for (soff, slen) in SBLK:
                            ps = pjp.tile([128, 512], f32, tag="pj", name="ps_p")
                            for c in range(4):
                                nc.tensor.matmul(ps[:, :slen], pts[c][:, m * 128:(m + 1) * 128],
                                   aoutT[c][:, soff:soff + slen],
                                   start=(c == 0), stop=(c == 3))
                            t = scr.tile([128, 512], f32, tag="rs", name="prs")
                            nc.vector.tensor_scalar(t[:, :slen], xtm[:, soff:soff + slen],
                                                    0.25, sv[:, m:m + 1], A.mult, A.add)
                            nc.vector.tensor_add(art[:, soff:soff + slen], t[:, :slen],
                                                 ps[:, :slen])
                        nc.sync.dma_start(ar_in[m * 128:(m + 1) * 128, :], art)
                if no_cc:
                    nc.sync.dma_start(ar_out, ar_in)
                else:
                    nc.gpsimd.collective_compute(
                        "AllReduce", A.add, replica_groups=[[0, 1, 2, 3], [4, 5, 6, 7]],
                        ins=[ar_in.opt()], outs=[ar_out.opt()])

                ao_es.close()

                if phases <= 6:
                    _close_stacks()
                    continue
                # ---------------- phase 7/8: LN2 + hidden-sharded MLP ----------------
                x2p = top.enter_context(tc.tile_pool(name="x2p", bufs=1))
                x2T = []
                for c in range(NK):
                    t = x2p.tile([128, S], f32, tag=f"x2T{c}", name=f"x2T{c}")
                    nc.sync.dma_start(t, ar_out[c * 128:(c + 1) * 128, :])
                    x2T.append(t)
                with ExitStack() as ph8:
                    lp = ph8.enter_context(tc.tile_pool(name="lp", bufs=1))
                    ln2T = layer_norm_T(x2T, lp, 'norm2_w', 'norm2_b', 'l2T', out_f32r=True)
                    hT = [lp.tile([128, S], f32, tag=f"hT{c}", name=f"hT{c}")
                          for c in range(NK)]

                    def ev_h(m, soff, slen, ps):
                        dst = hT[m][:, soff:soff + slen]
                        if not sim_gelu:
                            nc.scalar.activation(dst.bitcast(f32r), ps[:, :slen], AF.Gelu,
                                                 bias=vcol('fc1_b', m))
                            return
                        # CoreSim has no Gelu LUT: tanh-approx composition (sim only)
                        nc.scalar.activation(dst, ps[:, :slen], AF.Identity,
                                             bias=vcol('fc1_b', m))
                        s1 = scr.tile([128, 512], f32, tag="gl1", name="s1")[:, :slen]
                        nc.scalar.activation(s1, dst, AF.Square)
                        nc.vector.tensor_scalar(s1, s1, 0.044715, 1.0, A.mult, A.add)
                        nc.vector.tensor_mul(s1, s1, dst)
                        nc.vector.tensor_scalar_mul(s1, s1, 0.7978845608028654)
                        nc.scalar.activation(s1, s1, AF.Tanh)
                        nc.vector.tensor_scalar(s1, s1, 1.0, 0.5, A.add, A.mult)
                        nc.vector.tensor_mul(dst, dst, s1)
                    gemm_T(fc1T, FFH, ln2T, 0, S, ev_h, "w1")

                    with tc.tile_pool(name="yp", bufs=2) as yp:
                        ytiles = {}

                        def ev_y(m, soff, slen, ps):
                            if m not in ytiles:
                                ytiles[m] = yp.tile([128, S], f32, tag="yt", name="yt")
                            t = scr.tile([128, 512], f32, tag="rs", name="yrs")
                            nc.vector.tensor_scalar(t[:, :slen], x2T[m][:, soff:soff + slen],
                                                    0.25, sv[:, 8 + m:9 + m], A.mult, A.add)
                            nc.vector.tensor_add(ytiles[m][:, soff:soff + slen], t[:, :slen],
                                                 ps[:, :slen])
                            if soff + slen >= S:
                                nc.sync.dma_start(y_t[m * 128:(m + 1) * 128, :], ytiles[m])
                        gemm_T(fc2T, D, hT, 0, S, ev_y, "w2")

    nc.compile()
    return nc


def _prep_in_maps(inputs):
    def c(a):
        return np.ascontiguousarray(np.asarray(a), dtype=np.float32)

    def rnd(a):
        # round to fp32r-representable values (zero low mantissa bits)
        return ((a.view(np.uint32) + 0x8000) & 0xFFFF0000).view(np.float32)

    qkv_w = np.asarray(inputs['qkv_w'])
    qkv_b = np.asarray(inputs['qkv_b'])
    in_maps = []
    for core in range(8):
        b, g = core // 4, core % 4
        hs = slice(4 * g * HD, 4 * g * HD + DHC)
        v = np.zeros((128, NV), np.float32)
        for k in ('norm1_w', 'norm1_b', 'rq_b', 'rk_b', 'proj_b',
                  'norm2_w', 'norm2_b', 'fc2_b'):
            arr = np.asarray(inputs[k])
            v[:, VOFF[k]:VOFF[k] + 8] = arr.reshape(8, 128).T
        v[:, VOFF['fc1_b']:VOFF['fc1_b'] + 8] = \
            np.asarray(inputs['fc1_b'])[FFH * g:FFH * (g + 1)].reshape(8, 128).T
        v[:, VOFF['qkv_bq']:VOFF['qkv_bq'] + 2] = qkv_b[0:D][hs].reshape(2, 128).T
        v[:, VOFF['qkv_bk']:VOFF['qkv_bk'] + 2] = qkv_b[D:2 * D][hs].reshape(2, 128).T
        v[:, VOFF['qkv_bv']:VOFF['qkv_bv'] + 2] = qkv_b[2 * D:][hs].reshape(2, 128).T
        in_maps.append({
            'x_t': c(np.asarray(inputs['x'])[b].T),
            'rq_wT': c(np.asarray(inputs['rq_w']).T),
            'rk_wT': c(np.asarray(inputs['rk_w']).T),
            'pos_bias': c(inputs['pos_bias']),
            'wqT': c(qkv_w[0:D][hs, :].T),
            'wkT': c(qkv_w[D:2 * D][hs, :].T),
            'wvT': c(qkv_w[2 * D:][hs, :].T),
            'projT': c(np.asarray(inputs['proj_w'])[:, hs].T),
            'fc1T': rnd(c(np.asarray(inputs['fc1_w'])[FFH * g:FFH * (g + 1), :].T)),
            'fc2T': rnd(c(np.asarray(inputs['fc2_w'])[:, FFH * g:FFH * (g + 1)].T)),
            'vecs': c(v),
        })
    return in_maps


def get_nc(sim_gelu=False, reps=1, no_cc=False, phases=99):
    key = f'nc{sim_gelu}_{reps}_{no_cc}_{phases}'
    if key not in _CACHE:
        _CACHE[key] = build_nc(sim_gelu, reps, no_cc, phases)
    return _CACHE[key]


def assemble(results):
    out = np.zeros((B, S, D), np.float32)
    for b in range(2):
        acc = np.zeros((D, S), np.float64)
        for c in range(4 * b, 4 * b + 4):
            acc += results[c]['y_t']
        out[b] = acc.T.astype(np.float32)
    return out


def kernel(**inputs):
    from concourse.bass_utils import run_bass_kernel_spmd
    nc = get_nc()
    in_maps = _prep_in_maps(inputs)
    res = run_bass_kernel_spmd(nc, in_maps, list(range(8))).results
    return assemble(res)

